# revision 52
# baseline (speedup 1.0000x reference)
"""Trainium2 Bass kernel for nn_EulerFullAttention (v4).

Math (per batch b, head h, dh=64):
  theta_q = x/(1+|w_q|) + b_q + t*phi_q ; Q = [cos(theta_q), sin(theta_q)]  (S,128)
  theta_k likewise ; K = [cos, sin]
  V = cos(theta_v)+sin(theta_v) = sqrt(2)*sin(theta_v + pi/4)              (S,64)
  scores = Q @ K^T / sqrt(128), causal softmax, out = attn @ V
  result = sqrt(2)*sin(theta_o + pi/4), theta_o = out/(1+|w_out|) + b_out

Distribution: 8 cores = 2 batches x 4 head-groups (4 heads each). Each core
computes its x[:, 256-col] slice end to end; no collectives.

Host precomputes range-reduced phases in fp16 (O(S*D) prep):
  fq[s,v,d] = wrap(x*sq + bq/2pi + t*phi/2pi + 0.25*(v==0)) in [-0.5, 0.5]
  fk likewise ; fv[s,d] = wrap(x*sv + bv/2pi + 0.125)
Device phase A: one PE transpose per (head, proj, s-block) moves fq/fk
[128s, 2v x 64d] -> [128(v,d), 128s] fp16 PSUM; ACT Sin(2pi f) -> QT/KT
fp16 [128, S] (rows 0:64 cos, 64:128 sin). V = ACT Sin of fv scattered
straight into vaug[128, head, block, 65] (col 64 = ones -> denominator).

Phase B (qc-outer desc, head-inner): scoresT[k, q] = KT_blk^T @ QT (fp16
matmul, fp32 PSUM, packs of [128,1024], 3 rotating tiles); causal masking
is an additive -30000 upper-triangle matmul accumulated into the diagonal
strips (ident^T @ mneg) so exp needs no post-mask; ACT Exp -> ext fp16;
attn@V with vaug stationary / ext moving accumulates ot[65, 512] per
512-q chunk (row 64 = softmax denominator, single accumulation group:
PSUM start zeroes the whole 2KB zero-region so start/stop appear exactly
once). DVE copies ot -> SBUF fp16, PE transposes back to q-major
[128, 4, 65] psum, fp16 reciprocal + one broadcast multiply write onat.
PE work is software-pipelined: scores of pack p+1 are emitted before
attn@V of pack p so the in-order PE queue never head-of-line blocks.

Phase C: ro = onat * sqrt(2)/(1+|w_out|); ACT Sin(ro + pi/4 + b_out) --
input range [-0.63, 2.2] is inside the Sin table domain (+-3.8 measured)
so no range reduction. Host multiplies the fp16 result by sqrt(2).

All ACT instructions are dependency-chained in emission order so the tile
scheduler cannot interleave Sin and Exp -> exactly 3 act-table loads.
"""

import sys, math

sys.path.insert(0, "/opt/trn_rl_repo")

import numpy as np
import concourse.bass as bass
import concourse.mybir as mybir
from concourse.bacc import Bacc
from concourse.tile import TileContext
from concourse.bass_utils import run_bass_kernel_spmd
from contextlib import ExitStack

F32 = mybir.dt.float32
F16 = mybir.dt.float16
AF = mybir.ActivationFunctionType
ALU = mybir.AluOpType

B, S, D, H = 2, 2048, 1024, 16
DH = 64
NH = 4            # heads per core
DC = NH * DH      # 256 feature columns per core
NB = S // 128     # 16 s-blocks
TWO_PI = 2.0 * math.pi
SQRT2 = math.sqrt(2.0)
EXP_SCALE = 1.0 / math.sqrt(2.0 * DH)
ACT_CHAIN = "boundary"


def _bcast_mid(ap2d, n):
    """[128, F] AP -> [128, n, F] with stride-0 middle dim."""
    return bass.AP(tensor=ap2d.tensor, offset=ap2d.offset,
                   ap=[ap2d.ap[0], [0, n], ap2d.ap[-1]])


def _swap12(ap4):
    """Reorder dims 1,2 of a 4D AP (iteration order change only)."""
    return bass.AP(tensor=ap4.tensor, offset=ap4.offset,
                   ap=[ap4.ap[0], ap4.ap[2], ap4.ap[1], ap4.ap[3]])


def _build_packs(qc):
    """PSUM pack layout for one 512-wide q chunk: list of packs, each a list
    of (kb, qs, N, off) strips placed in a [128,1024] (2-bank) psum tile."""
    order = list(range(4 * qc)) + [4 * qc, 4 * qc + 1, 4 * qc + 3, 4 * qc + 2]
    packs, cur, off = [], [], 0
    for kb in order:
        if kb < 4 * qc:
            qs, N = 512 * qc, 512
        else:
            jj = kb - 4 * qc
            qs, N = 512 * qc + 128 * jj, 512 - 128 * jj
        o = off
        if o % 512 + N > 512:
            o = (o // 512 + 1) * 512
        if o + N > 1024:
            packs.append(cur)
            cur, o = [], 0
        cur.append((kb, qs, N, o))
        off = o + N
    if cur:
        packs.append(cur)
    return packs


def build_nc():
    nc = Bacc(trn_type="TRN2")
    fq_d = nc.dram_tensor("fq", [NH, S, DH], F16, kind="ExternalInput")
    fk_d = nc.dram_tensor("fk", [NH, S, DH], F16, kind="ExternalInput")
    fv_d = nc.dram_tensor("fv", [S, DC], F16, kind="ExternalInput")
    op_d = nc.dram_tensor("oprm", [128, DC], F16, kind="ExternalInput")
    bias_d = nc.dram_tensor("obias", [128, 1], F32, kind="ExternalInput")
    out_d = nc.dram_tensor("out", [S, DC], F16, kind="ExternalOutput")
    _k = np.arange(128)[:, None]
    _q = np.arange(128)[None, :]
    _im = np.concatenate([np.eye(128, dtype=np.float16),
                          np.where(_q >= _k, np.float16(0.0),
                                   np.float16(-30000.0))], axis=1)
    im_d = nc.inline_tensor(_im, "identmneg")

    acts = []  # activation handles in required engine order

    with TileContext(nc) as tc, ExitStack() as ctx:
        sing = ctx.enter_context(tc.tile_pool(name="sing", bufs=1))
        qkpool = ctx.enter_context(tc.tile_pool(name="qkp", bufs=1))
        expool = ctx.enter_context(tc.tile_pool(name="exp", bufs=6))
        otsp = ctx.enter_context(tc.tile_pool(name="otsp", bufs=2))
        obufp = ctx.enter_context(tc.tile_pool(name="obuf", bufs=4))
        tiny = ctx.enter_context(tc.tile_pool(name="tiny", bufs=4))
        psp = ctx.enter_context(tc.tile_pool(name="psp", bufs=3, space="PSUM"))
        pso = ctx.enter_context(tc.tile_pool(name="pso", bufs=1, space="PSUM"))
        psn = ctx.enter_context(tc.tile_pool(name="psn", bufs=1, space="PSUM"))

        # ---------------- input DMA ----------------
        fq_s = sing.tile([128, NH, NB, DH], F16)
        fk_s = sing.tile([128, NH, NB, DH], F16)
        fv_s = sing.tile([128, NB, DC], F16)
        fq_r = fq_d[:, :, :].rearrange("j (n p) d -> p j n d", p=128)
        fk_r = fk_d[:, :, :].rearrange("j (n p) d -> p j n d", p=128)
        fv_r = fv_d[:, :].rearrange("(n p) d -> p n d", p=128)
        im_s = sing.tile([128, 256], F16)
        nc.sync.dma_start(out=im_s, in_=im_d[:, :])
        ident = im_s[:, 0:128]
        mneg = im_s[:, 128:256]
        nc.sync.dma_start(out=fq_s[:, 0, :, :], in_=fq_r[:, 0, :, :])
        nc.sync.dma_start(out=fk_s[:, 0, :, :], in_=fk_r[:, 0, :, :])
        nc.sync.dma_start(out=fv_s[:, 0:4, :], in_=fv_r[:, 0:4, :])
        for j in range(1, NH):
            nc.sync.dma_start(out=fq_s[:, j, :, :], in_=fq_r[:, j, :, :])
            nc.sync.dma_start(out=fk_s[:, j, :, :], in_=fk_r[:, j, :, :])
            sl = slice(4 * j, 4 * j + 4)
            nc.sync.dma_start(out=fv_s[:, sl, :], in_=fv_r[:, sl, :])
        oprm = sing.tile([128, DC], F16)
        nc.sync.dma_start(out=oprm, in_=op_d[:, :])
        obias = sing.tile([128, 1], F32)
        nc.sync.dma_start(out=obias, in_=bias_d[:, :])
        bz = sing.tile([128, 1], F32)
        nc.vector.memset(bz, 0.0)
        bqk = sing.tile([128, 1], F32)
        nc.vector.memset(bqk[0:64, :], math.pi / 4)
        nc.vector.memset(bqk[64:128, :], -math.pi / 4)

        vaug = sing.tile([128, NH, NB, DH + 1], F16)
        nc.vector.memset(vaug[:, :, :, DH:DH + 1], 1.0)
        onat = sing.tile([128, NB, DC], F16)

        QT, KT = [], []
        for j in range(NH):
            qt_j = qkpool.tile([128, S], F16, tag=f"q{j}")
            kt_j = qkpool.tile([128, S], F16, tag=f"k{j}")
            QT.append(qt_j)
            KT.append(kt_j)

        # ---------------- phase A: sins ----------------
        def qk_prep(j, pi):
            src = fq_s if pi == 0 else fk_s
            dst = QT[j] if pi == 0 else KT[j]
            xtp_raw = psp.tile([128, 1024], F32, tag="ps")
            xtp = xtp_raw[:, 0:1024].bitcast(F16)
            for n in range(NB):
                blk = src[:, j, n, :]
                nc.tensor.transpose(xtp[0:64, 128 * n:128 * n + 128], blk, ident)
                nc.tensor.transpose(xtp[64:128, 128 * n:128 * n + 128], blk, ident)
            acts.append(nc.scalar.activation(
                out=dst[:, :], in_=xtp, func=AF.Sin, bias=bqk[:, 0:1],
                scale=TWO_PI))

        def v_quarter(qq):
            in_ap = bass.AP(
                tensor=fv_s.tensor, offset=fv_s.offset + 4 * qq * DC,
                ap=[fv_s.ap[0], [DC, 4], [DH, NH], [1, DH]])
            sl4 = vaug[:, :, 4 * qq:4 * qq + 4, 0:DH]
            acts.append(nc.scalar.activation(
                out=_swap12(sl4), in_=in_ap, func=AF.Sin,
                bias=bz[:, 0:1], scale=TWO_PI))

        for j in range(NH):
            qk_prep(j, 0)
            qk_prep(j, 1)
            v_quarter(j)

        # ---------------- phase B: attention (software-pipelined) ----------
        # stream of (j, qc, pack, first, last)
        stream = []
        for qc in (3, 2, 1, 0):
            for j in range(NH):
                packs = _build_packs(qc)
                for i, pack in enumerate(packs):
                    stream.append((j, qc, pack, i == 0, i == len(packs) - 1))

        state = {}          # (j, qc) -> dict(ot=..., avi=..., n_av=...)
        ro_tiles = {}       # qc -> ro tile ready for the final sin
        pending_v = None    # closure: emit attn@V for the previous pack
        pending_t = []      # deferred transpose+normalize closures

        def make_v(j, qc, pack, first, last, ext):
            def emit():
                st = state.get((j, qc))
                if st is None:
                    ot = pso.tile([65, 512], F32, tag="po")
                    st = state[(j, qc)] = {
                        "ot": ot, "avi": 0,
                        "n_av": sum(1 for p in _build_packs(qc) for _ in p)}
                ot = st["ot"]
                for (kb, qs, N, off) in pack:
                    q0 = qs - 512 * qc
                    st["avi"] += 1
                    nc.tensor.matmul(ot[:, q0:q0 + N],
                                     vaug[:, j, kb, :],
                                     ext[:, off:off + N],
                                     start=(st["avi"] == 1),
                                     stop=(st["avi"] == st["n_av"]))
                if last:
                    ot_s = otsp.tile([65, 512], F16, tag="ots")
                    nc.vector.tensor_copy(out=ot_s, in_=ot)

                    def post():
                        onp = psn.tile([128, 4, DH + 2], F16, tag="pn")
                        for t4 in range(4):
                            nc.tensor.transpose(
                                onp[:, t4, 0:DH + 1],
                                ot_s[:, 128 * t4:128 * t4 + 128],
                                ident[0:65, 0:65])
                        rec = tiny.tile([128, 4, 1], F16, tag="tiny")
                        with nc.allow_low_precision(reason="softmax denom fp16"):
                            nc.vector.reciprocal(out=rec, in_=onp[:, :, DH:DH + 1])
                        nc.vector.tensor_tensor(
                            out=onat[:, 4 * qc:4 * qc + 4, DH * j:DH * j + DH],
                            in0=onp[:, :, 0:DH],
                            in1=bass.AP(tensor=rec.tensor, offset=rec.offset,
                                        ap=[rec.ap[0], rec.ap[1], [0, DH]]),
                            op=ALU.mult)
                        if j == NH - 1:  # quarter complete: prep final input
                            ro = obufp.tile([128, 4, DC], F16, tag="ob")
                            nc.vector.tensor_tensor(
                                out=ro, in0=onat[:, 4 * qc:4 * qc + 4, :],
                                in1=_bcast_mid(oprm[:, :], 4), op=ALU.mult)
                            ro_tiles[qc] = ro
                    pending_t.append(post)
            return emit

        for (j, qc, pack, first, last) in stream:
            # scores for this pack; diagonal strips get an additive causal
            # mask (-30000 above the diagonal) accumulated in the same group
            sc = psp.tile([128, 1024], F32, tag="ps")
            for (kb, qs, N, off) in pack:
                diag = kb >= 4 * qc
                nc.tensor.matmul(sc[:, off:off + N],
                                 KT[j][:, 128 * kb:128 * kb + 128],
                                 QT[j][:, qs:qs + N],
                                 start=True, stop=not diag)
                if diag:
                    nc.tensor.matmul(sc[:, off:off + 128], ident, mneg,
                                     start=False, stop=True)
            # attn@V of the previous pack (PE stays a pack behind exp)
            if pending_v is not None:
                pending_v()
                pending_v = None
            while len(pending_t) > (2 if qc > 0 else 0):
                pending_t.pop(0)()
            # exp + causal mask for this pack
            width = pack[-1][3] + pack[-1][2]
            ext = expool.tile([128, 1024], F16, tag="ex")
            acts.append(nc.scalar.activation(
                out=ext[:, 0:width], in_=sc[:, 0:width],
                func=AF.Exp, bias=bz[:, 0:1], scale=EXP_SCALE))
            pending_v = make_v(j, qc, pack, first, last, ext)

        pending_v()
        while pending_t:
            pending_t.pop(0)()

        # ---------------- phase C: final layer ----------------
        out_r = out_d[:, :].rearrange("(n p) d -> p n d", p=128)
        for qq in (3, 2, 1, 0):  # qc 0 finishes last; its quarter goes last
            ro = ro_tiles[qq]
            acts.append(nc.scalar.activation(out=ro, in_=ro, func=AF.Sin,
                                             bias=obias[:, 0:1], scale=1.0))
            nc.sync.dma_start(out=out_r[:, 4 * qq:4 * qq + 4, :], in_=ro)

        # pin ACT engine order so Sin/Exp phases never interleave.
        # ACT_CHAIN: "full" chains every pair (robust, ~130ns/act sem cost),
        # "boundary" pins only the phase transitions.
        if ACT_CHAIN == "full":
            for a, b in zip(acts[1:], acts[:-1]):
                bass._add_dep_helper(a.ins, b.ins, sync=True, reason="act-order")
        else:
            n_sins = 3 * NH  # qk sins + v sins emitted in phase A
            first_exp = acts[n_sins]
            for s in acts[:n_sins]:
                bass._add_dep_helper(first_exp.ins, s.ins, sync=True,
                                     reason="act-order")
            for f in acts[-4:]:
                bass._add_dep_helper(f.ins, acts[-5].ins, sync=True,
                                     reason="act-order")

    nc.finalize()
    return nc


def _host_params(inputs, c):
    """Per-core input dict for core c: precompute wrapped phases in fp16."""
    b, g = c // 4, c % 4
    inv2pi = 1.0 / (2.0 * np.pi)
    x = np.asarray(inputs["x"], dtype=np.float64)[b, :, DC * g:DC * g + DC]  # [S, DC]
    s_arr = np.arange(S, dtype=np.float64)[:, None]                          # [S, 1]

    def f64(a):
        return np.asarray(a, dtype=np.float64)

    def wrap(v):
        return (v + 0.5) % 1.0 - 0.5

    hsl = slice(NH * g, NH * g + NH)

    def phases(wn, bn, pn):
        w = f64(inputs[wn])[hsl].reshape(-1)[None, :]      # [1, DC]
        bb = f64(inputs[bn])[hsl].reshape(-1)[None, :]
        ph = f64(inputs[pn])[hsl].reshape(-1)[None, :]
        base = x * (inv2pi / (1.0 + np.abs(w))) + bb * inv2pi + s_arr * (ph * inv2pi)
        base = base.reshape(S, NH, DH).transpose(1, 0, 2)   # [NH, S, DH]
        # +0.125 shift: device Sin uses bias +pi/4 (cos rows) / -pi/4 (sin
        # rows), keeping |input| <= 3.93 rad, inside the usable table range
        return wrap(base + 0.125).astype(np.float16)

    fq = phases("w_q", "b_q", "phi_q")
    fk = phases("w_k", "b_k", "phi_k")

    wv = f64(inputs["w_v"])[hsl].reshape(-1)[None, :]
    bv = f64(inputs["b_v"])[hsl].reshape(-1)[None, :]
    fv = wrap(x * (inv2pi / (1.0 + np.abs(wv))) + bv * inv2pi + 0.125).astype(
        np.float16)

    wo = f64(inputs["w_out"])[DC * g:DC * g + DC]
    oprm = np.broadcast_to((SQRT2 / (1.0 + np.abs(wo)))[None, :],
                           (128, DC)).astype(np.float16)

    bo = f64(inputs["b_out"])
    assert np.all(bo == bo[0]), "non-uniform b_out unsupported"
    obias = np.full((128, 1), bo[0] + np.pi / 4, dtype=np.float32)

    return {"fq": fq, "fk": fk, "fv": fv, "oprm": np.ascontiguousarray(oprm),
            "obias": obias}


_NC_CACHE = {}


def kernel(**inputs) -> np.ndarray:
    in_maps = [_host_params(inputs, c) for c in range(8)]
    if "nc" not in _NC_CACHE:
        _NC_CACHE["nc"] = build_nc()
    nc = _NC_CACHE["nc"]
    res = run_bass_kernel_spmd(nc, in_maps, core_ids=list(range(8)))
    full = np.empty((B, S, D), dtype=np.float32)
    for c in range(8):
        b, g = c // 4, c % 4
        full[b, :, DC * g:DC * g + DC] = \
            np.asarray(res.results[c]["out"]).astype(np.float32) * SQRT2
    return full


# revision 53
# speedup vs baseline: 1.0046x; 1.0046x over previous
"""Trainium2 Bass kernel for nn_EulerFullAttention (v4).

Math (per batch b, head h, dh=64):
  theta_q = x/(1+|w_q|) + b_q + t*phi_q ; Q = [cos(theta_q), sin(theta_q)]  (S,128)
  theta_k likewise ; K = [cos, sin]
  V = cos(theta_v)+sin(theta_v) = sqrt(2)*sin(theta_v + pi/4)              (S,64)
  scores = Q @ K^T / sqrt(128), causal softmax, out = attn @ V
  result = sqrt(2)*sin(theta_o + pi/4), theta_o = out/(1+|w_out|) + b_out

Distribution: 8 cores = 2 batches x 4 head-groups (4 heads each). Each core
computes its x[:, 256-col] slice end to end; no collectives.

Host precomputes range-reduced phases in fp16 (O(S*D) prep):
  fq[s,v,d] = wrap(x*sq + bq/2pi + t*phi/2pi + 0.25*(v==0)) in [-0.5, 0.5]
  fk likewise ; fv[s,d] = wrap(x*sv + bv/2pi + 0.125)
Device phase A: one PE transpose per (head, proj, s-block) moves fq/fk
[128s, 2v x 64d] -> [128(v,d), 128s] fp16 PSUM; ACT Sin(2pi f) -> QT/KT
fp16 [128, S] (rows 0:64 cos, 64:128 sin). V = ACT Sin of fv scattered
straight into vaug[128, head, block, 65] (col 64 = ones -> denominator).

Phase B (qc-outer desc, head-inner): scoresT[k, q] = KT_blk^T @ QT (fp16
matmul, fp32 PSUM, packs of [128,1024], 3 rotating tiles); causal masking
is an additive -30000 upper-triangle matmul accumulated into the diagonal
strips (ident^T @ mneg) so exp needs no post-mask; ACT Exp -> ext fp16;
attn@V with vaug stationary / ext moving accumulates ot[65, 512] per
512-q chunk (row 64 = softmax denominator, single accumulation group:
PSUM start zeroes the whole 2KB zero-region so start/stop appear exactly
once). DVE copies ot -> SBUF fp16, PE transposes back to q-major
[128, 4, 65] psum, fp16 reciprocal + one broadcast multiply write onat.
PE work is software-pipelined: scores of pack p+1 are emitted before
attn@V of pack p so the in-order PE queue never head-of-line blocks.

Phase C: ro = onat * sqrt(2)/(1+|w_out|); ACT Sin(ro + pi/4 + b_out) --
input range [-0.63, 2.2] is inside the Sin table domain (+-3.8 measured)
so no range reduction. Host multiplies the fp16 result by sqrt(2).

All ACT instructions are dependency-chained in emission order so the tile
scheduler cannot interleave Sin and Exp -> exactly 3 act-table loads.
"""

import sys, math

sys.path.insert(0, "/opt/trn_rl_repo")

import numpy as np
import concourse.bass as bass
import concourse.mybir as mybir
from concourse.bacc import Bacc
from concourse.tile import TileContext
from concourse.bass_utils import run_bass_kernel_spmd
from contextlib import ExitStack

F32 = mybir.dt.float32
F16 = mybir.dt.float16
AF = mybir.ActivationFunctionType
ALU = mybir.AluOpType

B, S, D, H = 2, 2048, 1024, 16
DH = 64
NH = 4            # heads per core
DC = NH * DH      # 256 feature columns per core
NB = S // 128     # 16 s-blocks
TWO_PI = 2.0 * math.pi
SQRT2 = math.sqrt(2.0)
EXP_SCALE = 1.0 / math.sqrt(2.0 * DH)
ACT_CHAIN = "boundary"


def _bcast_mid(ap2d, n):
    """[128, F] AP -> [128, n, F] with stride-0 middle dim."""
    return bass.AP(tensor=ap2d.tensor, offset=ap2d.offset,
                   ap=[ap2d.ap[0], [0, n], ap2d.ap[-1]])


def _swap12(ap4):
    """Reorder dims 1,2 of a 4D AP (iteration order change only)."""
    return bass.AP(tensor=ap4.tensor, offset=ap4.offset,
                   ap=[ap4.ap[0], ap4.ap[2], ap4.ap[1], ap4.ap[3]])


def _build_packs(qc):
    """PSUM pack layout for one 512-wide q chunk: list of packs, each a list
    of (kb, qs, N, off) strips placed in a [128,1024] (2-bank) psum tile."""
    order = list(range(4 * qc)) + [4 * qc, 4 * qc + 1, 4 * qc + 3, 4 * qc + 2]
    packs, cur, off = [], [], 0
    for kb in order:
        if kb < 4 * qc:
            qs, N = 512 * qc, 512
        else:
            jj = kb - 4 * qc
            qs, N = 512 * qc + 128 * jj, 512 - 128 * jj
        o = off
        if o % 512 + N > 512:
            o = (o // 512 + 1) * 512
        if o + N > 1024:
            packs.append(cur)
            cur, o = [], 0
        cur.append((kb, qs, N, o))
        off = o + N
    if cur:
        packs.append(cur)
    return packs


def build_nc():
    nc = Bacc(trn_type="TRN2")
    fq_d = nc.dram_tensor("fq", [NH, S, DH], F16, kind="ExternalInput")
    fk_d = nc.dram_tensor("fk", [NH, S, DH], F16, kind="ExternalInput")
    fv_d = nc.dram_tensor("fv", [S, DC], F16, kind="ExternalInput")
    op_d = nc.dram_tensor("oprm", [128, DC], F16, kind="ExternalInput")
    bias_d = nc.dram_tensor("obias", [128, 1], F32, kind="ExternalInput")
    out_d = nc.dram_tensor("out", [S, DC], F16, kind="ExternalOutput")
    _k = np.arange(128)[:, None]
    _q = np.arange(128)[None, :]
    _im = np.concatenate([np.eye(128, dtype=np.float16),
                          np.where(_q >= _k, np.float16(0.0),
                                   np.float16(-30000.0))], axis=1)
    im_d = nc.inline_tensor(_im, "identmneg")

    acts = []  # activation handles in required engine order

    with TileContext(nc) as tc, ExitStack() as ctx:
        sing = ctx.enter_context(tc.tile_pool(name="sing", bufs=1))
        qkpool = ctx.enter_context(tc.tile_pool(name="qkp", bufs=1))
        expool = ctx.enter_context(tc.tile_pool(name="exp", bufs=6))
        otsp = ctx.enter_context(tc.tile_pool(name="otsp", bufs=2))
        obufp = ctx.enter_context(tc.tile_pool(name="obuf", bufs=4))
        tiny = ctx.enter_context(tc.tile_pool(name="tiny", bufs=4))
        psp = ctx.enter_context(tc.tile_pool(name="psp", bufs=3, space="PSUM"))
        pso = ctx.enter_context(tc.tile_pool(name="pso", bufs=1, space="PSUM"))
        psn = ctx.enter_context(tc.tile_pool(name="psn", bufs=1, space="PSUM"))

        # ---------------- input DMA ----------------
        fq_s = sing.tile([128, NH, NB, DH], F16)
        fk_s = sing.tile([128, NH, NB, DH], F16)
        fv_s = sing.tile([128, NB, DC], F16)
        fq_r = fq_d[:, :, :].rearrange("j (n p) d -> p j n d", p=128)
        fk_r = fk_d[:, :, :].rearrange("j (n p) d -> p j n d", p=128)
        fv_r = fv_d[:, :].rearrange("(n p) d -> p n d", p=128)
        im_s = sing.tile([128, 256], F16)
        nc.sync.dma_start(out=im_s, in_=im_d[:, :])
        ident = im_s[:, 0:128]
        mneg = im_s[:, 128:256]
        nc.sync.dma_start(out=fq_s[:, 0, :, :], in_=fq_r[:, 0, :, :])
        nc.sync.dma_start(out=fk_s[:, 0, :, :], in_=fk_r[:, 0, :, :])
        nc.sync.dma_start(out=fv_s[:, 0:4, :], in_=fv_r[:, 0:4, :])
        for j in range(1, NH):
            nc.sync.dma_start(out=fq_s[:, j, :, :], in_=fq_r[:, j, :, :])
            nc.sync.dma_start(out=fk_s[:, j, :, :], in_=fk_r[:, j, :, :])
            sl = slice(4 * j, 4 * j + 4)
            nc.sync.dma_start(out=fv_s[:, sl, :], in_=fv_r[:, sl, :])
        oprm = sing.tile([128, DC], F16)
        nc.sync.dma_start(out=oprm, in_=op_d[:, :])
        obias = sing.tile([128, 1], F32)
        nc.sync.dma_start(out=obias, in_=bias_d[:, :])
        bz = sing.tile([128, 1], F32)
        nc.vector.memset(bz, 0.0)
        bqk = sing.tile([128, 1], F32)
        nc.vector.memset(bqk[0:64, :], math.pi / 4)
        nc.vector.memset(bqk[64:128, :], -math.pi / 4)

        # PE p-state warmup: ~40 dummy matmuls from t~0.7us ramp the PE to
        # full clock before the first real transposes (saves ~4us of
        # cold-clock transpose time at the head of phase A)
        warm = psp.tile([128, 1024], F32, tag="ps")
        for _ in range(40):
            nc.tensor.matmul(warm[:, 0:128], ident, ident,
                             start=True, stop=True)
        warm_s = sing.tile([128, 128], F32)
        nc.vector.tensor_copy(out=warm_s, in_=warm[:, 0:128])

        vaug = sing.tile([128, NH, NB, DH + 1], F16)
        nc.vector.memset(vaug[:, :, :, DH:DH + 1], 1.0)
        onat = sing.tile([128, NB, DC], F16)

        QT, KT = [], []
        for j in range(NH):
            qt_j = qkpool.tile([128, S], F16, tag=f"q{j}")
            kt_j = qkpool.tile([128, S], F16, tag=f"k{j}")
            QT.append(qt_j)
            KT.append(kt_j)

        # ---------------- phase A: sins ----------------
        def qk_prep(j, pi):
            src = fq_s if pi == 0 else fk_s
            dst = QT[j] if pi == 0 else KT[j]
            xtp_raw = psp.tile([128, 1024], F32, tag="ps")
            xtp = xtp_raw[:, 0:1024].bitcast(F16)
            for n in range(NB):
                blk = src[:, j, n, :]
                nc.tensor.transpose(xtp[0:64, 128 * n:128 * n + 128], blk, ident)
                nc.tensor.transpose(xtp[64:128, 128 * n:128 * n + 128], blk, ident)
            acts.append(nc.scalar.activation(
                out=dst[:, :], in_=xtp, func=AF.Sin, bias=bqk[:, 0:1],
                scale=TWO_PI))

        def v_quarter(qq):
            in_ap = bass.AP(
                tensor=fv_s.tensor, offset=fv_s.offset + 4 * qq * DC,
                ap=[fv_s.ap[0], [DC, 4], [DH, NH], [1, DH]])
            sl4 = vaug[:, :, 4 * qq:4 * qq + 4, 0:DH]
            acts.append(nc.scalar.activation(
                out=_swap12(sl4), in_=in_ap, func=AF.Sin,
                bias=bz[:, 0:1], scale=TWO_PI))

        for j in range(NH):
            qk_prep(j, 0)
            qk_prep(j, 1)
            v_quarter(j)

        # ---------------- phase B: attention (software-pipelined) ----------
        # stream of (j, qc, pack, first, last)
        stream = []
        for qc in (3, 2, 1, 0):
            for j in range(NH):
                packs = _build_packs(qc)
                for i, pack in enumerate(packs):
                    stream.append((j, qc, pack, i == 0, i == len(packs) - 1))

        state = {}          # (j, qc) -> dict(ot=..., avi=..., n_av=...)
        ro_tiles = {}       # qc -> ro tile ready for the final sin
        pending_v = None    # closure: emit attn@V for the previous pack
        pending_t = []      # deferred transpose+normalize closures

        def make_v(j, qc, pack, first, last, ext):
            def emit():
                st = state.get((j, qc))
                if st is None:
                    ot = pso.tile([65, 512], F32, tag="po")
                    st = state[(j, qc)] = {
                        "ot": ot, "avi": 0,
                        "n_av": sum(1 for p in _build_packs(qc) for _ in p)}
                ot = st["ot"]
                for (kb, qs, N, off) in pack:
                    q0 = qs - 512 * qc
                    st["avi"] += 1
                    nc.tensor.matmul(ot[:, q0:q0 + N],
                                     vaug[:, j, kb, :],
                                     ext[:, off:off + N],
                                     start=(st["avi"] == 1),
                                     stop=(st["avi"] == st["n_av"]))
                if last:
                    ot_s = otsp.tile([65, 512], F16, tag="ots")
                    nc.vector.tensor_copy(out=ot_s, in_=ot)

                    def post():
                        onp = psn.tile([128, 4, DH + 2], F16, tag="pn")
                        for t4 in range(4):
                            nc.tensor.transpose(
                                onp[:, t4, 0:DH + 1],
                                ot_s[:, 128 * t4:128 * t4 + 128],
                                ident[0:65, 0:65])
                        rec = tiny.tile([128, 4, 1], F16, tag="tiny")
                        with nc.allow_low_precision(reason="softmax denom fp16"):
                            nc.vector.reciprocal(out=rec, in_=onp[:, :, DH:DH + 1])
                        nc.vector.tensor_tensor(
                            out=onat[:, 4 * qc:4 * qc + 4, DH * j:DH * j + DH],
                            in0=onp[:, :, 0:DH],
                            in1=bass.AP(tensor=rec.tensor, offset=rec.offset,
                                        ap=[rec.ap[0], rec.ap[1], [0, DH]]),
                            op=ALU.mult)
                        if j == NH - 1:  # quarter complete: prep final input
                            ro = obufp.tile([128, 4, DC], F16, tag="ob")
                            nc.vector.tensor_tensor(
                                out=ro, in0=onat[:, 4 * qc:4 * qc + 4, :],
                                in1=_bcast_mid(oprm[:, :], 4), op=ALU.mult)
                            ro_tiles[qc] = ro
                    pending_t.append(post)
            return emit

        for (j, qc, pack, first, last) in stream:
            # scores for this pack; diagonal strips get an additive causal
            # mask (-30000 above the diagonal) accumulated in the same group
            sc = psp.tile([128, 1024], F32, tag="ps")
            for (kb, qs, N, off) in pack:
                diag = kb >= 4 * qc
                nc.tensor.matmul(sc[:, off:off + N],
                                 KT[j][:, 128 * kb:128 * kb + 128],
                                 QT[j][:, qs:qs + N],
                                 start=True, stop=not diag)
                if diag:
                    nc.tensor.matmul(sc[:, off:off + 128], ident, mneg,
                                     start=False, stop=True)
            # attn@V of the previous pack (PE stays a pack behind exp)
            if pending_v is not None:
                pending_v()
                pending_v = None
            while len(pending_t) > (2 if qc > 0 else 0):
                pending_t.pop(0)()
            # exp + causal mask for this pack
            width = pack[-1][3] + pack[-1][2]
            ext = expool.tile([128, 1024], F16, tag="ex")
            acts.append(nc.scalar.activation(
                out=ext[:, 0:width], in_=sc[:, 0:width],
                func=AF.Exp, bias=bz[:, 0:1], scale=EXP_SCALE))
            pending_v = make_v(j, qc, pack, first, last, ext)

        pending_v()
        while pending_t:
            pending_t.pop(0)()

        # ---------------- phase C: final layer ----------------
        out_r = out_d[:, :].rearrange("(n p) d -> p n d", p=128)
        for qq in (3, 2, 1, 0):  # qc 0 finishes last; its quarter goes last
            ro = ro_tiles[qq]
            acts.append(nc.scalar.activation(out=ro, in_=ro, func=AF.Sin,
                                             bias=obias[:, 0:1], scale=1.0))
            nc.sync.dma_start(out=out_r[:, 4 * qq:4 * qq + 4, :], in_=ro)

        # pin ACT engine order so Sin/Exp phases never interleave.
        # ACT_CHAIN: "full" chains every pair (robust, ~130ns/act sem cost),
        # "boundary" pins only the phase transitions.
        if ACT_CHAIN == "full":
            for a, b in zip(acts[1:], acts[:-1]):
                bass._add_dep_helper(a.ins, b.ins, sync=True, reason="act-order")
        else:
            n_sins = 3 * NH  # qk sins + v sins emitted in phase A
            first_exp = acts[n_sins]
            for s in acts[:n_sins]:
                bass._add_dep_helper(first_exp.ins, s.ins, sync=True,
                                     reason="act-order")
            for f in acts[-4:]:
                bass._add_dep_helper(f.ins, acts[-5].ins, sync=True,
                                     reason="act-order")

    nc.finalize()
    return nc


def _host_params(inputs, c):
    """Per-core input dict for core c: precompute wrapped phases in fp16."""
    b, g = c // 4, c % 4
    inv2pi = 1.0 / (2.0 * np.pi)
    x = np.asarray(inputs["x"], dtype=np.float64)[b, :, DC * g:DC * g + DC]  # [S, DC]
    s_arr = np.arange(S, dtype=np.float64)[:, None]                          # [S, 1]

    def f64(a):
        return np.asarray(a, dtype=np.float64)

    def wrap(v):
        return (v + 0.5) % 1.0 - 0.5

    hsl = slice(NH * g, NH * g + NH)

    def phases(wn, bn, pn):
        w = f64(inputs[wn])[hsl].reshape(-1)[None, :]      # [1, DC]
        bb = f64(inputs[bn])[hsl].reshape(-1)[None, :]
        ph = f64(inputs[pn])[hsl].reshape(-1)[None, :]
        base = x * (inv2pi / (1.0 + np.abs(w))) + bb * inv2pi + s_arr * (ph * inv2pi)
        base = base.reshape(S, NH, DH).transpose(1, 0, 2)   # [NH, S, DH]
        # +0.125 shift: device Sin uses bias +pi/4 (cos rows) / -pi/4 (sin
        # rows), keeping |input| <= 3.93 rad, inside the usable table range
        return wrap(base + 0.125).astype(np.float16)

    fq = phases("w_q", "b_q", "phi_q")
    fk = phases("w_k", "b_k", "phi_k")

    wv = f64(inputs["w_v"])[hsl].reshape(-1)[None, :]
    bv = f64(inputs["b_v"])[hsl].reshape(-1)[None, :]
    fv = wrap(x * (inv2pi / (1.0 + np.abs(wv))) + bv * inv2pi + 0.125).astype(
        np.float16)

    wo = f64(inputs["w_out"])[DC * g:DC * g + DC]
    oprm = np.broadcast_to((SQRT2 / (1.0 + np.abs(wo)))[None, :],
                           (128, DC)).astype(np.float16)

    bo = f64(inputs["b_out"])
    assert np.all(bo == bo[0]), "non-uniform b_out unsupported"
    obias = np.full((128, 1), bo[0] + np.pi / 4, dtype=np.float32)

    return {"fq": fq, "fk": fk, "fv": fv, "oprm": np.ascontiguousarray(oprm),
            "obias": obias}


_NC_CACHE = {}


def kernel(**inputs) -> np.ndarray:
    in_maps = [_host_params(inputs, c) for c in range(8)]
    if "nc" not in _NC_CACHE:
        _NC_CACHE["nc"] = build_nc()
    nc = _NC_CACHE["nc"]
    res = run_bass_kernel_spmd(nc, in_maps, core_ids=list(range(8)))
    full = np.empty((B, S, D), dtype=np.float32)
    for c in range(8):
        b, g = c // 4, c % 4
        full[b, :, DC * g:DC * g + DC] = \
            np.asarray(res.results[c]["out"]).astype(np.float32) * SQRT2
    return full
